# revision 1
# baseline (speedup 1.0000x reference)
"""Causal self-attention (B=4, S=2048, D=1024, H=16, hd=64) on 8 TRN2 NeuronCores.

Sharding: core c handles batch b = c//2 and head-half g = c%2 (8 heads, 512 of
the 1024 qkv dims).  Each core computes its partial output projection
(x[b] @ Wq_g.T ... attention ... @ Wp_g.T); the host sums the two partials per
batch and adds the bias.

Device kernel (per core), all matmuls in float32r (TF32-like, full-rate):
  P1: Q^T, K^T ([hd, S] layout) and V ([S, hd] layout, with a ones-column per
      head for the softmax denominator) projections from x^T.
  P2: flash-style attention per head-pair chunk: scores computed transposed
      (S^T = K_h @ Q_h^T tiles, [k x q]), exp on ScalarE (scale=1/8 folded in),
      causal masking via gpsimd affine_select on diagonal tiles only, AV
      matmuls accumulate y^T and the softmax denominator (ones column) in
      PSUM; normalization via reciprocal + a small broadcast matmul.
  P3: output projection -> partial out^T, DMA to DRAM.
"""

import numpy as np

B, S, D, H, HD = 4, 2048, 1024, 16, 64
N_CORES = 8
LH = H // 2          # local heads per core (8)
P = 128


def _ensure_concourse():
    try:
        import concourse  # noqa: F401
    except ImportError:
        import sys
        for p in ("/opt/trn_rl_repo", "/root/.axon_site/_ro/trn_rl_repo"):
            if p not in sys.path:
                sys.path.append(p)
        import concourse  # noqa: F401


def build_nc(S_=S, D_=D, LH_=LH, num_devices=N_CORES):
    """Build the per-core Bass program.  Parameterized so a small config can be
    validated in CoreSim.  Requires S_%512==0, D_%128==0, LH_%2==0."""
    _ensure_concourse()
    import concourse.tile as tile
    from concourse import bacc, mybir

    f32 = mybir.dt.float32
    f32r = mybir.dt.float32r
    EXP = mybir.ActivationFunctionType.Exp
    MULT = mybir.AluOpType.mult
    IS_GE = mybir.AluOpType.is_ge

    LHD = LH_ * HD            # local head dims (512)
    NPAIR = LH_ // 2          # head pairs (4)
    DCH = D_ // P             # d contraction chunks (8)
    CH = LHD // P             # hd contraction chunks for out proj (4)
    NQT = S_ // 512           # q tiles (4)
    NKC = S_ // P             # k chunks (16)
    QT = 512                  # q tile width
    KC = P                    # k chunk width

    nc = bacc.Bacc("TRN2", target_bir_lowering=False, debug=False,
                   enable_asserts=True, num_devices=num_devices)

    xT = nc.dram_tensor("xT", [D_, S_], f32, kind="ExternalInput").ap()
    wqT = nc.dram_tensor("wqT", [D_, LHD], f32, kind="ExternalInput").ap()
    wkT = nc.dram_tensor("wkT", [D_, LHD], f32, kind="ExternalInput").ap()
    wvT = nc.dram_tensor("wvT", [D_, LHD], f32, kind="ExternalInput").ap()
    wpT = nc.dram_tensor("wpT", [LHD, D_], f32, kind="ExternalInput").ap()
    outT = nc.dram_tensor("outT", [D_, S_], f32, kind="ExternalOutput").ap()

    xT_r = xT.bitcast(f32r).rearrange("(ko p) s -> p ko s", p=P)
    wqT_r = wqT.bitcast(f32r).rearrange("(ko p) m -> p ko m", p=P)
    wkT_r = wkT.bitcast(f32r).rearrange("(ko p) m -> p ko m", p=P)
    wvT_r = wvT.bitcast(f32r).rearrange("(ko p) m -> p ko m", p=P)
    wpT_r = wpT.bitcast(f32r).rearrange("(co p) d -> p co d", p=P)

    with tile.TileContext(nc) as tc:
        with tc.tile_pool(name="persist", bufs=1) as persist:
            # Persistent SBUF tensors.
            qT = persist.tile([P, NPAIR, S_], f32r, tag="qT")
            kT = persist.tile([P, NPAIR, S_], f32r, tag="kT")
            # v: [s-part, kchunk, head, 64 v-dims + ones col]
            v_sb = persist.tile([P, NKC, LH_, HD + 1], f32r, tag="v")
            # selector for the denominator broadcast matmul (rows 64, 96).
            # memset can't write f32r (invalid ISA); stage in f32 and copy.
            sel = persist.tile([P, P], f32r, tag="sel")
            rt = persist.tile([P, QT], f32r, tag="rt")
            scr = persist.tile([P, QT], f32, tag="scr")
            nc.vector.memset(scr[:], 0.0)
            nc.vector.memset(scr[64:65, 0:64], 1.0)
            nc.vector.memset(scr[96:97, 64:128], 1.0)
            with nc.allow_low_precision(reason="0/1 selector exact in f32r"):
                nc.vector.tensor_copy(sel[:], scr[:, 0:P])
                nc.vector.tensor_copy(rt[64:97, :], scr[64:97, :])
            # ones column of v: fill everything with 1.0; the V projection
            # copies overwrite cols 0..63 of each head block, leaving col 64
            nc.vector.memset(v_sb[:].rearrange("p a b c -> p (a b c)").bitcast(f32), 1.0)

            # ---------------- P1: projections ----------------
            with tc.tile_pool(name="xw", bufs=1) as xw, \
                 tc.tile_pool(name="wstr", bufs=2) as wstr, \
                 tc.tile_pool(name="ps1", bufs=2, space="PSUM") as ps1:
                # a=0 weight DMAs first so the first matmuls aren't behind
                # the bulk x transfer in the DMA queues
                wq0 = wstr.tile([P, DCH, P], f32r, tag="wq")
                wk0 = wstr.tile([P, DCH, P], f32r, tag="wk")
                nc.sync.dma_start(wq0[:], wqT_r[:, :, 0:P])
                nc.sync.dma_start(wk0[:], wkT_r[:, :, 0:P])
                # x in quarter chunks, n-major; wv right after the first
                # quarter so V isn't stuck behind the bulk x transfer
                xt = xw.tile([P, DCH, S_], f32r, tag="xt")
                for k in range(DCH):
                    nc.sync.dma_start(xt[:, k, 0:QT], xT_r[:, k, 0:QT])
                wv_t = xw.tile([P, DCH, LHD], f32r, tag="wv")
                for k in range(DCH):
                    nc.sync.dma_start(wv_t[:, k, :], wvT_r[:, k, :])
                for n in range(1, NQT):
                    for k in range(DCH):
                        nc.sync.dma_start(xt[:, k, n * QT:(n + 1) * QT],
                                          xT_r[:, k, n * QT:(n + 1) * QT])

                # V projection: out[s-chunk 128, LHD]
                for s in range(NKC):
                    psv = ps1.tile([P, LHD], f32, tag="psv")
                    for k in range(DCH):
                        nc.tensor.matmul(psv[:], xt[:, k, s * P:(s + 1) * P],
                                         wv_t[:, k, :],
                                         start=(k == 0), stop=(k == DCH - 1))
                    # scatter into per-head 65-wide blocks (cols 0..63)
                    nc.vector.tensor_copy(v_sb[:, s, :, 0:HD],
                                          psv.rearrange("p (h d) -> p h d", d=HD))

                # Q^T / K^T projections: out[hd-chunk 128, q-tile 512]
                for a in range(NPAIR):
                    if a == 0:
                        wq_t, wk_t = wq0, wk0
                    else:
                        wq_t = wstr.tile([P, DCH, P], f32r, tag="wq")
                        wk_t = wstr.tile([P, DCH, P], f32r, tag="wk")
                        nc.sync.dma_start(wq_t[:], wqT_r[:, :, a * P:(a + 1) * P])
                        nc.sync.dma_start(wk_t[:], wkT_r[:, :, a * P:(a + 1) * P])
                    for n in range(NQT):
                        psq = ps1.tile([P, QT], f32, tag="psq")
                        psk = ps1.tile([P, QT], f32, tag="psk")
                        for k in range(DCH):
                            nc.tensor.matmul(psq[:], wq_t[:, k, :],
                                             xt[:, k, n * QT:(n + 1) * QT],
                                             start=(k == 0), stop=(k == DCH - 1))
                        for k in range(DCH):
                            nc.tensor.matmul(psk[:], wk_t[:, k, :],
                                             xt[:, k, n * QT:(n + 1) * QT],
                                             start=(k == 0), stop=(k == DCH - 1))
                        nc.vector.tensor_copy(qT[:, a, n * QT:(n + 1) * QT], psq[:])
                        nc.vector.tensor_copy(kT[:, a, n * QT:(n + 1) * QT], psk[:])

            # ---------------- P2 + P3 ----------------
            with tc.tile_pool(name="ysb", bufs=1) as ysb, \
                 tc.tile_pool(name="ppool", bufs=4) as ppool, \
                 tc.tile_pool(name="small", bufs=2) as small, \
                 tc.tile_pool(name="ostg", bufs=2) as ostg, \
                 tc.tile_pool(name="ps_s", bufs=2, space="PSUM") as ps_s, \
                 tc.tile_pool(name="ps_y", bufs=3, space="PSUM") as ps_y, \
                 tc.tile_pool(name="ps_m", bufs=1, space="PSUM") as ps_m:
                yT = ysb.tile([P, CH, S_], f32r, tag="yT")
                wp_t = ysb.tile([P, CH, D_], f32r, tag="wp")
                for c in range(CH):
                    nc.sync.dma_start(wp_t[:, c, :], wpT_r[:, c, :])

                for j in range(NQT):
                    kcount = 4 * j + 4
                    for a in range(NPAIR):
                        psA = ps_y.tile([P, QT], f32, tag="psy")
                        psB = ps_y.tile([P, QT], f32, tag="psy")
                        # diagonal (masked) k-chunks first so the gpsimd
                        # mask latency overlaps later unmasked chunks
                        for idx, kc in enumerate(range(kcount - 1, -1, -1)):
                            ss = ps_s.tile([P, 2 * QT], f32, tag="ss")
                            for h2 in range(2):
                                o = 64 * h2
                                nc.tensor.matmul(
                                    ss[:, h2 * QT:(h2 + 1) * QT],
                                    kT[o:o + 64, a, kc * KC:(kc + 1) * KC],
                                    qT[o:o + 64, a, j * QT:(j + 1) * QT],
                                    start=True, stop=True)
                            pt = ppool.tile([P, 2 * QT], f32r, tag="pt")
                            nc.scalar.activation(pt[:], ss[:], EXP, scale=0.125)
                            r = kc - 4 * j
                            if r >= 0:  # diagonal tile: causal mask
                                for h2 in range(2):
                                    nc.gpsimd.affine_select(
                                        out=pt[:, h2 * QT:(h2 + 1) * QT],
                                        in_=pt[:, h2 * QT:(h2 + 1) * QT],
                                        pattern=[[1, QT]],
                                        compare_op=IS_GE,
                                        fill=0.0,
                                        base=-KC * r,
                                        channel_multiplier=-1)
                            for h2, psy in ((0, psA), (1, psB)):
                                nc.tensor.matmul(
                                    psy[0:HD + 1, :],
                                    v_sb[:, kc, 2 * a + h2, :],
                                    pt[:, h2 * QT:(h2 + 1) * QT],
                                    start=(idx == 0), stop=(idx == kcount - 1))
                        # normalization (plain reciprocal; approx_fast is
                        # broken on HW in this env, and recip can't read PSUM)
                        with nc.allow_low_precision(
                                reason="softmax denominators in f32r"):
                            nc.vector.tensor_copy(rt[64:65, :], psA[HD:HD + 1, :])
                            nc.vector.tensor_copy(rt[96:97, :], psB[HD:HD + 1, :])
                            nc.vector.reciprocal(rt[64:65, :], rt[64:65, :])
                            nc.vector.reciprocal(rt[96:97, :], rt[96:97, :])
                        bc = ps_m.tile([P, QT], f32, tag="misc")
                        nc.tensor.matmul(bc[:], sel[64:97, :], rt[64:97, :],
                                         start=True, stop=True)
                        bcs = small.tile([P, QT], f32, tag="bcs")
                        nc.vector.tensor_copy(bcs[:], bc[:])
                        nc.vector.tensor_tensor(
                            yT[0:64, a, j * QT:(j + 1) * QT],
                            psA[0:HD, :], bcs[0:64, :], MULT)
                        nc.vector.tensor_tensor(
                            yT[64:128, a, j * QT:(j + 1) * QT],
                            psB[0:HD, :], bcs[64:128, :], MULT)

                    # P3 for this q tile (on the last tile, alternate PSUM
                    # slots with the now-idle attention pool to pipeline)
                    for m in range(DCH):
                        if j == NQT - 1 and m % 2 == 1:
                            po = ps_y.tile([P, QT], f32, tag="psy",
                                           name=f"po_{j}_{m}")
                        else:
                            po = ps_m.tile([P, QT], f32, tag="misc",
                                           name=f"po_{j}_{m}")
                        for c in range(CH):
                            nc.tensor.matmul(po[:], wp_t[:, c, m * P:(m + 1) * P],
                                             yT[:, c, j * QT:(j + 1) * QT],
                                             start=(c == 0), stop=(c == CH - 1))
                        ot = ostg.tile([P, QT], f32, tag="ot")
                        nc.vector.tensor_copy(ot[:], po[:])
                        nc.sync.dma_start(outT[m * P:(m + 1) * P,
                                               j * QT:(j + 1) * QT], ot[:])

    nc.compile()
    return nc


class _Runner:
    """Compile once; execute the SPMD program on 8 cores via PJRT."""

    def __init__(self):
        _ensure_concourse()
        import jax
        import numpy as _np
        from jax.sharding import Mesh, PartitionSpec
        from jax.experimental.shard_map import shard_map
        from concourse import bass2jax, mybir

        self.nc = build_nc()
        bass2jax.install_neuronx_cc_hook()
        nc = self.nc

        partition_name = (nc.partition_id_tensor.name
                          if nc.partition_id_tensor else None)
        in_names, out_names, out_avals, zero_shapes = [], [], [], []
        for alloc in nc.m.functions[0].allocations:
            if not isinstance(alloc, mybir.MemoryLocationSet):
                continue
            name = alloc.memorylocations[0].name
            if alloc.kind == "ExternalInput":
                if name != partition_name:
                    in_names.append(name)
            elif alloc.kind == "ExternalOutput":
                out_names.append(name)
                shape = tuple(alloc.tensor_shape)
                dtype = mybir.dt.np(alloc.dtype)
                out_avals.append(jax.core.ShapedArray(shape, dtype))
                zero_shapes.append((shape, dtype))
        self.in_names, self.out_names = in_names, out_names
        self.out_avals, self.zero_shapes = out_avals, zero_shapes
        n_params, n_outs = len(in_names), len(out_names)

        all_in_names = in_names + out_names
        if partition_name is not None:
            all_in_names = all_in_names + [partition_name]

        def _body(*args):
            operands = list(args)
            if partition_name is not None:
                operands.append(bass2jax.partition_id_tensor())
            outs = bass2jax._bass_exec_p.bind(
                *operands,
                out_avals=tuple(out_avals),
                in_names=tuple(all_in_names),
                out_names=tuple(out_names),
                lowering_input_output_aliases=(),
                sim_require_finite=True,
                sim_require_nnan=True,
                nc=nc,
            )
            return tuple(outs)

        devices = jax.devices()[:N_CORES]
        mesh = Mesh(_np.asarray(devices), ("core",))
        donate = tuple(range(n_params, n_params + n_outs))
        self._sharded = jax.jit(
            shard_map(_body, mesh=mesh,
                      in_specs=(PartitionSpec("core"),) * (n_params + n_outs),
                      out_specs=(PartitionSpec("core"),) * n_outs,
                      check_rep=False),
            donate_argnums=donate, keep_unused=True)

    def __call__(self, in_maps):
        import numpy as _np
        concat_in = [
            _np.concatenate([in_maps[c][name] for c in range(N_CORES)], axis=0)
            for name in self.in_names
        ]
        concat_zeros = [
            _np.zeros((N_CORES * s[0], *s[1:]), dt) for s, dt in self.zero_shapes
        ]
        out_arrs = self._sharded(*concat_in, *concat_zeros)
        return [
            {name: _np.asarray(out_arrs[i]).reshape(N_CORES, *self.out_avals[i].shape)[c]
             for i, name in enumerate(self.out_names)}
            for c in range(N_CORES)
        ]


_RUNNER = None


def _get_runner():
    global _RUNNER
    if _RUNNER is None:
        _RUNNER = _Runner()
    return _RUNNER


def shard_inputs(x, Wq, Wk, Wv, Wp):
    """Full inputs -> per-core input maps (host-side layout prep)."""
    in_maps = []
    for c in range(N_CORES):
        b, g = c // 2, c % 2
        sl = slice(g * LH * HD, (g + 1) * LH * HD)
        in_maps.append({
            "xT": np.ascontiguousarray(x[b].T),
            "wqT": np.ascontiguousarray(Wq[sl, :].T),
            "wkT": np.ascontiguousarray(Wk[sl, :].T),
            "wvT": np.ascontiguousarray(Wv[sl, :].T),
            "wpT": np.ascontiguousarray(Wp[:, sl].T),
        })
    return in_maps


def kernel(x, Wq, Wk, Wv, Wp, bp):
    x = np.asarray(x, dtype=np.float32)
    Wq = np.asarray(Wq, dtype=np.float32)
    Wk = np.asarray(Wk, dtype=np.float32)
    Wv = np.asarray(Wv, dtype=np.float32)
    Wp = np.asarray(Wp, dtype=np.float32)
    bp = np.asarray(bp, dtype=np.float32)

    runner = _get_runner()
    outs = runner(shard_inputs(x, Wq, Wk, Wv, Wp))
    out = np.empty((B, S, D), np.float32)
    for b in range(B):
        out[b] = outs[2 * b]["outT"].T + outs[2 * b + 1]["outT"].T + bp
    return out



# revision 5
# speedup vs baseline: 1.1798x; 1.1798x over previous
"""Causal self-attention (B=4, S=2048, D=1024, H=16, hd=64) on 8 TRN2 NeuronCores.

Sharding: core c handles batch b = c//2 and head-half g = c%2 (8 heads, 512 of
the 1024 qkv dims).  Each core computes its partial output projection; the host
sums the two partials per batch and adds the bias.

Device kernel (per core):
  P1: Q/K/V projections in bf16 from x^T (bf16).  psq/psk are cast to fp8e4m3,
      staged per head-pair, and round-tripped through DRAM to repack into the
      [32(p), half, head, s] layout needed for DoubleRow score matmuls.  V
      lands in SBUF as [128(k), kc, head, 65] bf16 with a ones column that
      produces softmax denominators inside the AV matmuls.
  P2: per (q-tile j, head-pair a): score tiles S^T = K_h^T Q_h [k, q] via fp8
      DoubleRow matmuls (0.5 cycles/row, windowed to the causal extent), exp
      on ScalarE -> bf16 probs, 0/1 triangular masking of the diagonal
      128x128 block on DVE, then flipped AV matmuls out[q, hd+1] with the
      probability block as the stationary operand: full 128-partition
      utilization and per-partition (per-q) softmax denominators.  Normalize
      with reciprocal + tensor_scalar into y8 [q, hd-pair] bf16, transpose on
      the PE (identity trick) and copy to yT[hd, q] for the out projection.
  P3: output projection in bf16 -> partial out^T (f32), one DMA per q tile.

Notes:
 - AV accumulators are pre-zeroed (memset) and accumulated with start=False:
   several independent accumulation groups share a 2KB PSUM zero region, so
   start=True would wipe sibling accumulators.  Each 65-wide output is given
   a 128-element stride so no matmul output crosses a PSUM bank boundary.
 - DMA instructions hold their issuing sequencer while waiting on data, so
   DMAs are few and bulk: 18 input loads, 8 repack stores, 16 repack loads,
   4 output stores, all on the SP queue.
"""

import numpy as np

B, S, D, H, HD = 4, 2048, 1024, 16, 64
N_CORES = 8
LH = H // 2          # local heads per core (8)
P = 128


def _ensure_concourse():
    try:
        import concourse  # noqa: F401
    except ImportError:
        import sys
        for p in ("/opt/trn_rl_repo", "/root/.axon_site/_ro/trn_rl_repo"):
            if p not in sys.path:
                sys.path.append(p)
        import concourse  # noqa: F401


def build_nc(S_=S, D_=D, LH_=LH, num_devices=N_CORES, upto='full'):
    """Per-core Bass program.  Requires S_%512==0, D_%256==0, LH_%2==0."""
    _ensure_concourse()
    import concourse.tile as tile
    from concourse import bacc, mybir

    f32 = mybir.dt.float32
    bf16 = mybir.dt.bfloat16
    f8 = mybir.dt.float8e4
    EXP = mybir.ActivationFunctionType.Exp
    MULT = mybir.AluOpType.mult
    IS_GE = mybir.AluOpType.is_ge
    DR = mybir.MatmulPerfMode.DoubleRow

    LHD = LH_ * HD            # local head dims (512)
    NPAIR = LH_ // 2          # head pairs (4)
    DCH = D_ // P             # d contraction chunks (8)
    CH = LHD // P             # hd contraction chunks for out proj (4)
    NQT = S_ // 512           # q tiles (4)
    NKC = S_ // P             # k chunks (16)
    QT = 512                  # q tile width
    KPT = QT // P             # k chunks per q tile (4)

    nc = bacc.Bacc("TRN2", target_bir_lowering=False, debug=False,
                   enable_asserts=True, num_devices=num_devices)

    # q/k/v projections run as 3-term fp8 hi/lo DoubleRow matmuls; x and the
    # qkv weights arrive pre-split (and weights pre-scaled by 32 against
    # e4m3 subnormal flush) from the host.
    xhi = nc.dram_tensor("xhi", [D_, S_], f8, kind="ExternalInput").ap()
    xlo = nc.dram_tensor("xlo", [D_, S_], f8, kind="ExternalInput").ap()
    wq8 = nc.dram_tensor("wq8", [2, D_, LHD], f8, kind="ExternalInput").ap()
    wk8 = nc.dram_tensor("wk8", [2, D_, LHD], f8, kind="ExternalInput").ap()
    wv8 = nc.dram_tensor("wv8", [2, D_, LHD], f8, kind="ExternalInput").ap()
    wpT = nc.dram_tensor("wpT", [LHD, D_], bf16, kind="ExternalInput").ap()
    outT = nc.dram_tensor("outT", [D_, S_], f32, kind="ExternalOutput").ap()

    DPR = DCH // 2            # paired d chunks for DoubleRow (4)
    xhi_r = xhi.rearrange("(c i p) s -> p c i s", p=P, i=2)
    xlo_r = xlo.rearrange("(c i p) s -> p c i s", p=P, i=2)
    wq8_r = wq8.rearrange("t (c i p) m -> t p c i m", p=P, i=2)
    wk8_r = wk8.rearrange("t (c i p) m -> t p c i m", p=P, i=2)
    wv8_r = wv8.rearrange("t (c i p) m -> t p c i m", p=P, i=2)
    wpT_r = wpT.rearrange("(co p) d -> p co d", p=P)
    outT_r = outT.rearrange("(mo p) s -> p mo s", p=P)

    with tile.TileContext(nc) as tc:
        with tc.tile_pool(name="persist", bufs=1) as persist, \
             tc.tile_pool(name="dram", bufs=1, space="DRAM") as dram:
            # fp8 q/k in DoubleRow layout, one tile per head pair so the
            # attention phase can start as soon as that pair's repack lands:
            # [32(p), half, h2, s]
            qT8 = [persist.tile([32, 2, 2, S_], f8, tag=f"qT8_{a}", name=f"qT8_{a}")
                   for a in range(NPAIR)]
            kT8 = [persist.tile([32, 2, 2, S_], f8, tag=f"kT8_{a}", name=f"kT8_{a}")
                   for a in range(NPAIR)]
            # v + ones column, one tile per k-chunk: [k-part, head, hd+1]
            v65 = [persist.tile([P, LH_, HD + 1], bf16, tag=f"v65_{s}", name=f"v65_{s}")
                   for s in range(NKC)]
            # y^T, one tile per q tile so the out projection overlaps P2
            yT = [persist.tile([P, CH, QT], bf16, tag=f"yT_{j}", name=f"yT_{j}")
                  for j in range(NQT)]
            wp_t = persist.tile([P, CH, D_], bf16, tag="wp")
            # -240 strictly above the diagonal (k>q), added to diagonal
            # score blocks before exp
            mneg = persist.tile([P, P], bf16, tag="mneg")
            ident = persist.tile([P, P], bf16, tag="ident")
            f32r = mybir.dt.float32r
            # denominator broadcast: sel2.T @ rt replicates the two
            # reciprocal rows to 64 partitions each (proven on-device
            # pattern; gpsimd partition_broadcast miscompiles to NaN here)
            sel2 = persist.tile([HD, P], f32r, tag="sel2")
            rt2 = persist.tile([HD, QT], f32r, tag="rt2")
            scr2 = persist.tile([HD, P], f32, tag="scr2")
            nc.vector.memset(scr2[:], 0.0)
            nc.vector.memset(scr2[0:1, 0:HD], 1.0)
            nc.vector.memset(scr2[32:33, HD:2 * HD], 1.0)
            with nc.allow_low_precision(reason="0/1 selector exact in f32r"):
                nc.vector.tensor_copy(sel2[:], scr2[:])
            nc.vector.memset(rt2.bitcast(f32), 0.0)
            # DRAM repack scratch, one tile per head pair: [ip(=32i+p32), h2, s]
            qdr = [dram.tile([64, 2, S_], f8, tag=f"qdr_{a}", name=f"qdr_{a}")
                   for a in range(NPAIR)]
            kdr = [dram.tile([64, 2, S_], f8, tag=f"kdr_{a}", name=f"kdr_{a}")
                   for a in range(NPAIR)]

            for s in range(NKC):
                nc.vector.memset(v65[s][:, :, HD:HD + 1], 1.0)
            nc.vector.memset(mneg[:], -240.0)
            nc.gpsimd.affine_select(
                out=mneg[:], in_=mneg[:],
                pattern=[[-1, P]], compare_op=IS_GE, fill=0.0,
                base=-1, channel_multiplier=1)
            # identity = lower-tri AND upper-tri of ones
            nc.vector.memset(ident[:], 1.0)
            nc.gpsimd.affine_select(out=ident[:], in_=ident[:],
                                    pattern=[[1, P]], compare_op=IS_GE,
                                    fill=0.0, base=0, channel_multiplier=-1)
            nc.gpsimd.affine_select(out=ident[:], in_=ident[:],
                                    pattern=[[-1, P]], compare_op=IS_GE,
                                    fill=0.0, base=0, channel_multiplier=1)

            # ---------------- single scope: no pool-release barriers ----
            with tc.tile_pool(name="xw", bufs=1) as xw, \
                 tc.tile_pool(name="wstr", bufs=2) as wstr, \
                 tc.tile_pool(name="stg", bufs=2) as stg, \
                 tc.tile_pool(name="ptp", bufs=5) as ptp, \
                 tc.tile_pool(name="nrm", bufs=2) as nrm, \
                 tc.tile_pool(name="otp", bufs=1) as otp, \
                 tc.tile_pool(name="p1p", bufs=2, space="PSUM") as p1p, \
                 tc.tile_pool(name="ps_s", bufs=2, space="PSUM") as ps_s, \
                 tc.tile_pool(name="ps_y", bufs=1, space="PSUM") as ps_y:
                # ---------------- P1: projections ----------------
                wq0 = wstr.tile([P, 2, DPR, 2, P], f8, tag="wq")
                wk0 = wstr.tile([P, 2, DPR, 2, P], f8, tag="wk")
                for t in range(2):
                    nc.sync.dma_start(wq0[:, t], wq8_r[t, :, :, :, 0:P])
                    nc.sync.dma_start(wk0[:, t], wk8_r[t, :, :, :, 0:P])
                x8 = xw.tile([P, 2, DPR, 2, S_], f8, tag="x8")
                # xhi first: the hi*hi / hi*lo terms only need xhi, so the
                # first projections start ~4us earlier
                for c in range(DPR):
                    nc.sync.dma_start(x8[:, 0, c], xhi_r[:, c, :, :])
                for c in range(DPR):
                    nc.sync.dma_start(x8[:, 1, c], xlo_r[:, c, :, :])
                # v/p weights go through the (otherwise idle) scalar-engine
                # DMA queue; wp is only needed by the out projection and is
                # loaded at the start of the last projection group
                wv_t = xw.tile([P, 2, DPR, 2, LHD], f8, tag="wv")
                for t in range(2):
                    nc.sync.dma_start(wv_t[:, t], wv8_r[t])

                # ---- emission blocks ----------------------------------
                # The Tile scheduler prioritizes by program order, so P1
                # (projections) and P2 (attention) are emitted interleaved:
                # attention for (q-tile j, head pair a) as soon as its q/k
                # repack and v chunks exist.  Otherwise the exp engine (the
                # eventual bottleneck) idles for the whole projection phase.
                # 3-term hi/lo products: (xhi*whi, xhi*wlo, xlo*whi)
                TERMS = ((0, 0), (0, 1), (1, 0))

                def emit_p1(a):
                    if a == 0:
                        wq_t, wk_t = wq0, wk0
                    else:
                        wq_t = wstr.tile([P, 2, DPR, 2, P], f8, tag="wq",
                                         name=f"wq_{a}")
                        wk_t = wstr.tile([P, 2, DPR, 2, P], f8, tag="wk",
                                         name=f"wk_{a}")
                        for t in range(2):
                            nc.sync.dma_start(
                                wq_t[:, t], wq8_r[t, :, :, :, a * P:(a + 1) * P])
                            nc.sync.dma_start(
                                wk_t[:, t], wk8_r[t, :, :, :, a * P:(a + 1) * P])
                    stq = stg.tile([P, NQT, QT], f8, tag="stq", name=f"stq_{a}")
                    stk = stg.tile([P, NQT, QT], f8, tag="stk", name=f"stk_{a}")
                    for n in range(NQT):
                        psq = p1p.tile([P, QT], f32, tag="p1",
                                       name=f"psq_{a}_{n}")
                        psk = p1p.tile([P, QT], f32, tag="p1",
                                       name=f"psk_{a}_{n}")
                        for w_t, ps in ((wq_t, psq), (wk_t, psk)):
                            first, last = (0, 0), (len(TERMS) - 1, DPR - 1)
                            for ti, (xt, wt) in enumerate(TERMS):
                                for c in range(DPR):
                                    nc.tensor.matmul(
                                        ps[:], w_t[:, wt, c, :, :],
                                        x8[:, xt, c, :, n * QT:(n + 1) * QT],
                                        start=((ti, c) == first),
                                        stop=((ti, c) == last),
                                        perf_mode=DR)
                        with nc.allow_low_precision(reason="fp8 scores"):
                            # 1/32 undoes the host-side weight prescale
                            nc.vector.tensor_scalar_mul(stq[:, n, :], psq[:],
                                                        1.0 / 32)
                            nc.vector.tensor_scalar_mul(stk[:, n, :], psk[:],
                                                        1.0 / 32)
                    # stores per (tensor, h2): [ip, s] -> qdr[a][ip, h2, s],
                    # then repack loads qdr[a][32i+p, h2, s] -> qT8[a][p,i,h2,s]
                    for h2 in range(2):
                        nc.sync.dma_start(
                            qdr[a][:, h2, :],
                            stq[64 * h2:64 * h2 + 64].rearrange("p n f -> p (n f)"))
                        nc.sync.dma_start(
                            kdr[a][:, h2, :],
                            stk[64 * h2:64 * h2 + 64].rearrange("p n f -> p (n f)"))
                    for h2 in range(2):
                        nc.sync.dma_start(
                            qT8[a][0:32, :, h2, :],
                            qdr[a][:, h2, :].rearrange("(i p) s -> p i s", i=2))
                        nc.sync.dma_start(
                            kT8[a][0:32, :, h2, :],
                            kdr[a][:, h2, :].rearrange("(i p) s -> p i s", i=2))
                def emit_p1v(s):
                    psv = p1p.tile([P, QT], f32, tag="p1",
                                   name=f"psv_{s}")[:, 0:LHD]
                    first, last = (0, 0), (len(TERMS) - 1, DPR - 1)
                    for ti, (xt, wt) in enumerate(TERMS):
                        for c in range(DPR):
                            nc.tensor.matmul(
                                psv, x8[:, xt, c, :, s * P:(s + 1) * P],
                                wv_t[:, wt, c, :, :],
                                start=((ti, c) == first),
                                stop=((ti, c) == last),
                                perf_mode=DR)
                    with nc.allow_low_precision(reason="bf16 v"):
                        nc.vector.tensor_scalar_mul(
                            v65[s][:, :, 0:HD],
                            psv.rearrange("p (h d) -> p h d", d=HD), 1.0 / 32)

                def make_av(j, a, kc, pt, ctx):
                    def emit():
                        # y^T orientation: out[hd+1, q] with v as stationary;
                        # row 64 accumulates the softmax denominators (ones
                        # column of v65).  One fat matmul per head instead of
                        # four narrow ones: fewer PE queue slots, no
                        # transposes afterwards.
                        r = kc - KPT * j
                        q0 = max(r, 0) * P
                        last = KPT * j + KPT - 1
                        for h2 in range(2):
                            nc.tensor.matmul(
                                ctx[h2][0:HD + 1, q0:QT],
                                v65[kc][:, 2 * a + h2, :],
                                pt[:, h2, q0:QT],
                                start=(kc == 0), stop=(kc == last),
                                skip_group_check=True)
                        if kc == last:
                            return [make_norm(j, a, ctx)]
                        return []
                    return emit

                def make_norm(j, a, ctx):
                    def emit():
                        psA, psB = ctx[0], ctx[1]
                        with nc.allow_low_precision(reason="softmax denom"):
                            nc.vector.tensor_copy(rt2[0:1, :],
                                                  psA[HD:HD + 1, :])
                            nc.vector.tensor_copy(rt2[32:33, :],
                                                  psB[HD:HD + 1, :])
                            nc.vector.reciprocal(rt2[0:1, :], rt2[0:1, :])
                            nc.vector.reciprocal(rt2[32:33, :], rt2[32:33, :])
                        bcp = p1p.tile([P, QT], f32, tag="p1",
                                       name=f"bcp_{j}_{a}")
                        nc.tensor.matmul(bcp[:], sel2[:], rt2[:],
                                         start=True, stop=True)
                        bcs = nrm.tile([P, QT], f32, tag="bc",
                                       name=f"bc_{j}_{a}")
                        nc.vector.tensor_copy(bcs[:], bcp[:])
                        with nc.allow_low_precision(reason="bf16 y"):
                            nc.vector.tensor_tensor(
                                yT[j][0:HD, a, :], psA[0:HD, :],
                                bcs[0:HD, :], MULT)
                            nc.vector.tensor_tensor(
                                yT[j][HD:2 * HD, a, :], psB[0:HD, :],
                                bcs[HD:2 * HD, :], MULT)
                    return emit

                # Deferred-emission queue: AV batches (and the transposes
                # they spawn) are emitted 2+ pipeline steps after their
                # producers, so by the time they reach the 4-deep PE wait
                # queue their pt/y8 dependencies are already satisfied and
                # they never head-of-line block the score matmuls that feed
                # the exp engine.
                stage_q = []
                bid_c = [0]

                def pump_keep(keep):
                    while len(stage_q) > keep:
                        bid, emit = stage_q.pop(0)
                        out = emit()
                        if out:
                            stage_q.extend((bid, f) for f in out)

                def pump_old(bid_limit):
                    while stage_q and stage_q[0][0] < bid_limit:
                        bid, emit = stage_q.pop(0)
                        out = emit()
                        if out:
                            # spawned work belongs to the same block and must
                            # flush with it (it reads the same psAV slot)
                            stage_q[0:0] = [(bid, f) for f in out]

                def emit_p2(j, a):
                    ctx = {}
                    bid = bid_c[0] = bid_c[0] + 1
                    for kc in range(KPT * j + KPT):
                        r = kc - KPT * j
                        q0 = max(r, 0) * P
                        ss = ps_s.tile([P, 2, QT], f32, tag="ss",
                                       name=f"ss_{j}_{a}_{kc}")
                        for h2 in range(2):
                            nc.tensor.matmul(
                                ss[:, h2, q0:QT],
                                kT8[a][:, :, h2, kc * P:(kc + 1) * P],
                                qT8[a][:, :, h2, j * QT + q0:(j + 1) * QT],
                                start=True, stop=(r < 0),
                                skip_group_check=True, perf_mode=DR)
                            if r >= 0:
                                # causal mask folded into the scores PSUM:
                                # add -240 above the diagonal of the 128x128
                                # diagonal block (exp then yields ~0 there)
                                nc.tensor.matmul(
                                    ss[:, h2, q0:q0 + P], ident[:], mneg[:],
                                    start=False, stop=True,
                                    skip_group_check=True)
                        pt = ptp.tile([P, 2, QT], bf16, tag="pt",
                                      name=f"pt_{j}_{a}_{kc}")
                        with nc.allow_low_precision(reason="bf16 probs"):
                            nc.scalar.activation(pt[:, :, q0:QT],
                                                 ss[:, :, q0:QT],
                                                 EXP, scale=0.125)
                        if upto == 'scores':
                            continue
                        if kc == 2:
                            # flush the previous block's leftovers (the
                            # single-buffered psY ring slots are recycled
                            # below) behind two steps of fresh scores
                            pump_old(bid)
                            ctx[0] = ps_y.tile([HD + 1, QT], f32, tag="psA",
                                               name=f"psA_{j}_{a}")
                            ctx[1] = ps_y.tile([HD + 1, QT], f32, tag="psB",
                                               name=f"psB_{j}_{a}")
                        elif kc > 2:
                            pump_keep(2)
                        stage_q.append((bid, make_av(j, a, kc, pt, ctx)))

                def emit_p3(j):
                    pump_keep(0)
                    otj = otp.tile([P, DCH, QT], f32, tag="otj",
                                   name=f"otj_{j}")
                    for m in range(DCH):
                        # the p1p psum pool is idle once projections finish
                        po = p1p.tile([P, QT], f32, tag="p1",
                                      name=f"po_{j}_{m}")
                        for c in range(CH):
                            nc.tensor.matmul(po[:], wp_t[:, c, m * P:(m + 1) * P],
                                             yT[j][:, c, :],
                                             start=(c == 0), stop=(c == CH - 1))
                        nc.vector.tensor_copy(otj[:, m, :], po[:])
                    nc.sync.dma_start(outT_r[:, :, j * QT:(j + 1) * QT],
                                      otj[:])

                for amax in range(NPAIR):
                    if amax == NPAIR - 1:
                        nc.sync.dma_start(wp_t[:], wpT_r[:])
                    emit_p1(amax)
                    if amax == 0:
                        # only the v chunks the first attention block needs;
                        # the rest are emitted after it so they don't hog the
                        # PE ahead of the first score matmuls
                        for s in range(KPT):
                            emit_p1v(s)
                    if upto == 'p1':
                        if amax == 0:
                            for s in range(KPT, NKC):
                                emit_p1v(s)
                        continue
                    last = (amax == NPAIR - 1)
                    if last:
                        js = list(range(NQT - 1, -1, -1))  # end on the shortest
                    else:
                        js = [(amax + t) % NQT for t in range(NQT)]
                    for t, j in enumerate(js):
                        emit_p2(j, amax)
                        if amax == 0 and t == 0:
                            for s in range(KPT, NKC):
                                emit_p1v(s)
                        if last and upto not in ('scores', 'av'):
                            emit_p3(j)

    nc.compile()
    return nc


class _Runner:
    """Compile once; execute the SPMD program on 8 cores via PJRT."""

    def __init__(self):
        _ensure_concourse()
        import jax
        import numpy as _np
        from jax.sharding import Mesh, PartitionSpec
        from jax.experimental.shard_map import shard_map
        from concourse import bass2jax, mybir

        self.nc = build_nc()
        bass2jax.install_neuronx_cc_hook()
        nc = self.nc

        partition_name = (nc.partition_id_tensor.name
                          if nc.partition_id_tensor else None)
        in_names, out_names, out_avals, zero_shapes = [], [], [], []
        for alloc in nc.m.functions[0].allocations:
            if not isinstance(alloc, mybir.MemoryLocationSet):
                continue
            name = alloc.memorylocations[0].name
            if alloc.kind == "ExternalInput":
                if name != partition_name:
                    in_names.append(name)
            elif alloc.kind == "ExternalOutput":
                out_names.append(name)
                shape = tuple(alloc.tensor_shape)
                dtype = mybir.dt.np(alloc.dtype)
                out_avals.append(jax.core.ShapedArray(shape, dtype))
                zero_shapes.append((shape, dtype))
        self.in_names, self.out_names = in_names, out_names
        self.out_avals, self.zero_shapes = out_avals, zero_shapes
        n_params, n_outs = len(in_names), len(out_names)

        all_in_names = in_names + out_names
        if partition_name is not None:
            all_in_names = all_in_names + [partition_name]

        def _body(*args):
            operands = list(args)
            if partition_name is not None:
                operands.append(bass2jax.partition_id_tensor())
            outs = bass2jax._bass_exec_p.bind(
                *operands,
                out_avals=tuple(out_avals),
                in_names=tuple(all_in_names),
                out_names=tuple(out_names),
                lowering_input_output_aliases=(),
                sim_require_finite=True,
                sim_require_nnan=True,
                nc=nc,
            )
            return tuple(outs)

        devices = jax.devices()[:N_CORES]
        mesh = Mesh(_np.asarray(devices), ("core",))
        donate = tuple(range(n_params, n_params + n_outs))
        self._sharded = jax.jit(
            shard_map(_body, mesh=mesh,
                      in_specs=(PartitionSpec("core"),) * (n_params + n_outs),
                      out_specs=(PartitionSpec("core"),) * n_outs,
                      check_rep=False),
            donate_argnums=donate, keep_unused=True)

    def __call__(self, in_maps):
        import numpy as _np
        concat_in = [
            _np.concatenate([in_maps[c][name] for c in range(N_CORES)], axis=0)
            for name in self.in_names
        ]
        concat_zeros = [
            _np.zeros((N_CORES * s[0], *s[1:]), dt) for s, dt in self.zero_shapes
        ]
        out_arrs = self._sharded(*concat_in, *concat_zeros)
        return [
            {name: _np.asarray(out_arrs[i]).reshape(N_CORES, *self.out_avals[i].shape)[c]
             for i, name in enumerate(self.out_names)}
            for c in range(N_CORES)
        ]


_RUNNER = None


def _get_runner():
    global _RUNNER
    if _RUNNER is None:
        _RUNNER = _Runner()
    return _RUNNER


def _hilo(m):
    """fp8 e4m3 hi/lo split."""
    import ml_dtypes
    e4 = ml_dtypes.float8_e4m3
    hi = m.astype(e4)
    lo = (m - hi.astype(np.float32)).astype(e4)
    return hi, lo


def shard_inputs(x, Wq, Wk, Wv, Wp):
    """Full inputs -> per-core input maps (host-side layout prep)."""
    import ml_dtypes
    bf = ml_dtypes.bfloat16
    in_maps = []
    for c in range(N_CORES):
        b, g = c // 2, c % 2
        sl = slice(g * LH * HD, (g + 1) * LH * HD)
        xhi, xlo = _hilo(np.ascontiguousarray(x[b].T))
        # weights prescaled by 32: N(0, 0.02) weights otherwise land in the
        # e4m3 subnormal range and lose most of their mantissa
        wq = np.stack(_hilo(np.ascontiguousarray(Wq[sl, :].T) * 32))
        wk = np.stack(_hilo(np.ascontiguousarray(Wk[sl, :].T) * 32))
        wv = np.stack(_hilo(np.ascontiguousarray(Wv[sl, :].T) * 32))
        in_maps.append({
            "xhi": xhi, "xlo": xlo,
            "wq8": wq, "wk8": wk, "wv8": wv,
            "wpT": np.ascontiguousarray(Wp[:, sl].T).astype(bf),
        })
    return in_maps


def kernel(x, Wq, Wk, Wv, Wp, bp):
    x = np.asarray(x, dtype=np.float32)
    Wq = np.asarray(Wq, dtype=np.float32)
    Wk = np.asarray(Wk, dtype=np.float32)
    Wv = np.asarray(Wv, dtype=np.float32)
    Wp = np.asarray(Wp, dtype=np.float32)
    bp = np.asarray(bp, dtype=np.float32)

    runner = _get_runner()
    outs = runner(shard_inputs(x, Wq, Wk, Wv, Wp))
    out = np.empty((B, S, D), np.float32)
    for b in range(B):
        out[b] = outs[2 * b]["outT"].T + outs[2 * b + 1]["outT"].T + bp
    return out
